# revision 7
# baseline (speedup 1.0000x reference)
"""DirectVoxGO Raw2Alpha + Alphas2Weights (segmented scan) on 8 Trainium2 cores.

Computes, for n_pts sample points sorted by ray_id:
    sp      = softplus(density + shift)
    log(1-alpha) = -interval * sp
    weights = alpha * T   (T = exclusive per-ray cumprod of (1-alpha))
    alphainv_last[r] = prod over ray r of (1-alpha)

Strategy
--------
Shard by ray: each of the 8 cores gets a contiguous chunk of points that
covers a contiguous range of rays (chunk boundaries snapped to ray starts,
host-side searchsorted).  Inside a core the chunk is split into 128
partition rows, each row also starting at a ray boundary, loaded via
indirect DMA (one row-start offset per partition).  Because every row
starts at a ray start, the per-ray segmented scan never crosses a
partition boundary and a single tensor_tensor_scan pass per column-tile
computes it:   state = m * state + sp    (m = 0 at ray starts).

The scan is linear in its data input, so the -interval factor is folded
into the Exp activations' scale.  softplus is computed as ln(1+exp(x))
(Softplus has no HW activation table; Exp and Ln share one set).

Row windows are a fixed width L >= every row's true length; the tail of a
window overlaps the next rows' points.  Values computed there are
bit-identical to the owning row's values (weights/expincl at a point only
depend on earlier points of its own ray, all inside the window), so the
overlapping scatter writes are harmless.

Outputs per core: weights (per point) and expincl = exp(-interval *
inclusive-scan) per point; the host gathers alphainv_last[r] =
expincl[last point of ray r] (empty rays -> 1.0).
"""

import functools

import numpy as np

P = 128           # SBUF partitions
NTILES = 8        # column tiles per row
N_CORES = 8


@functools.lru_cache(maxsize=4)
def _get_program(ft: int, npad: int, shift: float, interval: float, rid16: bool):
    import concourse.bass as bass
    import concourse.bacc as bacc
    import concourse.mybir as mybir
    import concourse.tile as tile

    AF = mybir.ActivationFunctionType
    OP = mybir.AluOpType
    f32 = mybir.dt.float32
    ridt = mybir.dt.int16 if rid16 else mybir.dt.int32
    i32 = mybir.dt.int32

    nc = bacc.Bacc("TRN2", target_bir_lowering=False, debug=False)
    dens = nc.dram_tensor("density_in", [npad, 1], f32, kind="ExternalInput")
    rid = nc.dram_tensor("rid_in", [npad, 1], ridt, kind="ExternalInput")
    offs = nc.dram_tensor("offs_in", [P, 1], i32, kind="ExternalInput")
    wout = nc.dram_tensor("weights_out", [npad, 1], f32, kind="ExternalOutput")
    eout = nc.dram_tensor("expincl_out", [npad, 1], f32, kind="ExternalOutput")

    with tile.TileContext(nc) as tc:
        with (
            tc.tile_pool(name="main", bufs=3) as pool,
            tc.tile_pool(name="chain", bufs=2) as chain_pool,
            tc.tile_pool(name="small", bufs=1) as spool,
        ):
            offs_sb = spool.tile([P, 1], i32)
            nc.sync.dma_start(out=offs_sb[:], in_=offs[:])
            shift_sb = spool.tile([P, 1], f32)
            nc.vector.memset(shift_sb[:], float(shift))

            r_prev = None
            incl_prev = None
            for t in range(NTILES):
                d_t = pool.tile([P, ft], f32, tag="d")
                r_t = chain_pool.tile([P, ft], ridt, tag="r")
                nc.gpsimd.indirect_dma_start(
                    out=d_t[:], out_offset=None, in_=dens[:],
                    in_offset=bass.IndirectOffsetOnAxis(ap=offs_sb[:, :1], axis=0),
                    element_offset=t * ft,
                )
                nc.gpsimd.indirect_dma_start(
                    out=r_t[:], out_offset=None, in_=rid[:],
                    in_offset=bass.IndirectOffsetOnAxis(ap=offs_sb[:, :1], axis=0),
                    element_offset=t * ft,
                )

                # softplus(x + shift) = ln(1 + exp(x + shift))
                # Exp in place on d (ACT -> ACT, same engine), then Ln to sp
                nc.scalar.activation(out=d_t[:], in_=d_t[:], func=AF.Exp,
                                     bias=shift_sb[:, :1], scale=1.0)
                sp_t = pool.tile([P, ft], f32, tag="sp")
                nc.scalar.activation(out=sp_t[:], in_=d_t[:], func=AF.Ln,
                                     bias=1.0, scale=1.0)

                # continue-mask: 1 where same ray as previous point, 0 at ray starts
                m_t = pool.tile([P, ft], f32, tag="m")
                nc.vector.tensor_tensor(out=m_t[:, 1:], in0=r_t[:, 1:],
                                        in1=r_t[:, : ft - 1], op=OP.is_equal)
                if t == 0:
                    # every row starts at a ray boundary
                    nc.vector.memset(m_t[:, 0:1], 0.0)
                else:
                    nc.vector.tensor_tensor(out=m_t[:, 0:1], in0=r_t[:, 0:1],
                                            in1=r_prev[:, ft - 1:ft],
                                            op=OP.is_equal)

                # segmented inclusive scan: state = m*state + sp
                incl_t = chain_pool.tile([P, ft], f32, tag="incl")
                nc.vector.tensor_tensor_scan(
                    out=incl_t[:], data0=m_t[:], data1=sp_t[:],
                    initial=(0.0 if t == 0 else incl_prev[:, ft - 1:ft]),
                    op0=OP.mult, op1=OP.add,
                )

                # excl = incl - sp  (exclusive scan)
                excl_t = pool.tile([P, ft], f32, tag="excl")
                nc.vector.tensor_sub(out=excl_t[:], in0=incl_t[:], in1=sp_t[:])
                # T = exp(-interval * excl)
                T_t = pool.tile([P, ft], f32, tag="T")
                nc.scalar.activation(out=T_t[:], in_=excl_t[:], func=AF.Exp,
                                     scale=-float(interval))
                # expincl = exp(-interval * incl)
                ei_t = pool.tile([P, ft], f32, tag="ei")
                nc.scalar.activation(out=ei_t[:], in_=incl_t[:], func=AF.Exp,
                                     scale=-float(interval))
                # weights = T - expincl = alpha * T
                w_t = pool.tile([P, ft], f32, tag="w")
                nc.vector.tensor_sub(out=w_t[:], in0=T_t[:], in1=ei_t[:])

                nc.gpsimd.indirect_dma_start(
                    out=wout[:],
                    out_offset=bass.IndirectOffsetOnAxis(ap=offs_sb[:, :1], axis=0),
                    in_=w_t[:], in_offset=None, element_offset=t * ft,
                )
                nc.gpsimd.indirect_dma_start(
                    out=eout[:],
                    out_offset=bass.IndirectOffsetOnAxis(ap=offs_sb[:, :1], axis=0),
                    in_=ei_t[:], in_offset=None, element_offset=t * ft,
                )
                r_prev, incl_prev = r_t, incl_t
    nc.compile()
    return nc


def _plan(rid: np.ndarray, n_rays: int, n_pts: int):
    """Host-side sharding: chunk + row boundaries, all snapped to ray starts."""
    rays_per_core = n_rays // N_CORES
    marks = np.arange(1, N_CORES) * rays_per_core
    bounds = np.concatenate(
        [[0], np.searchsorted(rid, marks, side="left"), [n_pts]]
    ).astype(np.int64)

    row_starts = []
    max_len = 0
    for k in range(N_CORES):
        b0, b1 = int(bounds[k]), int(bounds[k + 1])
        n_c = b1 - b0
        targets = b0 + (np.arange(P, dtype=np.int64) * n_c) // P
        ray_of_target = rid[np.minimum(targets, n_pts - 1)]
        s = np.searchsorted(rid, ray_of_target, side="left").astype(np.int64)
        s[0] = b0
        lens = np.diff(np.append(s, b1))
        max_len = max(max_len, int(lens.max()))
        row_starts.append(s)
    return bounds, row_starts, max_len


def kernel(density, shift, interval, ray_id, n_rays):
    from concourse.bass_utils import run_bass_kernel_spmd

    density = np.ascontiguousarray(np.asarray(density), dtype=np.float32).ravel()
    rid = np.ascontiguousarray(np.asarray(ray_id), dtype=np.int32).ravel()
    shift_f = float(np.asarray(shift))
    interval_f = float(np.asarray(interval))
    n_rays_i = int(np.asarray(n_rays))
    n_pts = density.shape[0]
    assert rid.shape[0] == n_pts and n_rays_i % N_CORES == 0

    bounds, row_starts, max_len = _plan(rid, n_rays_i, n_pts)

    # column-tile width: fits the longest row, multiple of 32 elements
    ft = -(-max_len // NTILES)
    ft = -(-ft // 32) * 32
    L = ft * NTILES

    # padded per-core length: every row window [s_p, s_p+L) must fit
    npad = 0
    for k in range(N_CORES):
        npad = max(npad, int(row_starts[k][-1] - bounds[k]) + L)
    npad = -(-npad // 32) * 32

    rid16 = n_rays_i <= 65536
    nc = _get_program(ft, npad, shift_f, interval_f, rid16)

    in_maps = []
    for k in range(N_CORES):
        b0, b1 = int(bounds[k]), int(bounds[k + 1])
        n_c = b1 - b0
        d_part = np.zeros((npad, 1), np.float32)
        d_part[:n_c, 0] = density[b0:b1]
        if rid16:
            r_part = np.zeros((npad, 1), np.int16)
            r_part[:n_c, 0] = rid[b0:b1].astype(np.uint16).view(np.int16)
        else:
            r_part = np.zeros((npad, 1), np.int32)
            r_part[:n_c, 0] = rid[b0:b1]
        offs_part = (row_starts[k] - b0).astype(np.int32).reshape(P, 1)
        assert int(offs_part.max()) + L <= npad
        in_maps.append(
            {"density_in": d_part, "rid_in": r_part, "offs_in": offs_part}
        )

    res = run_bass_kernel_spmd(nc, in_maps, list(range(N_CORES))).results

    weights = np.empty(n_pts, np.float32)
    expincl = np.empty(n_pts, np.float32)
    for k in range(N_CORES):
        b0, b1 = int(bounds[k]), int(bounds[k + 1])
        n_c = b1 - b0
        weights[b0:b1] = res[k]["weights_out"][:n_c, 0]
        expincl[b0:b1] = res[k]["expincl_out"][:n_c, 0]

    ray_ids = np.arange(n_rays_i)
    ends = np.searchsorted(rid, ray_ids, side="right") - 1
    starts = np.searchsorted(rid, ray_ids, side="left")
    alphainv_last = np.ones(n_rays_i, np.float32)
    nonempty = ends >= starts
    alphainv_last[nonempty] = expincl[ends[nonempty]]
    return weights, alphainv_last


# revision 8
# speedup vs baseline: 1261.3137x; 1261.3137x over previous
"""DirectVoxGO Raw2Alpha + Alphas2Weights (segmented scan) on 8 Trainium2 cores.

Computes, for n_pts sample points sorted by ray_id:
    sp      = softplus(density + shift)
    log(1-alpha) = -interval * sp
    weights = alpha * T   (T = exclusive per-ray cumprod of (1-alpha))
    alphainv_last[r] = prod over ray r of (1-alpha)

Strategy
--------
Shard by ray: each of the 8 cores gets a contiguous chunk of points that
covers a contiguous range of rays (chunk boundaries snapped to ray starts,
host-side searchsorted).  Inside a core the chunk is laid out as a
[128, L] grid; each partition row also starts at a ray boundary (host
marshals rows into the grid, padding short rows).  Because every row
starts at a ray start, the per-ray segmented scan never crosses a
partition boundary and a single tensor_tensor_scan pass per column tile
computes it:   state = m * state + sp    (m = 0 at ray starts).

The scan is linear in its data input, so the -interval factor is folded
into the Exp activations' scale.  softplus is computed as ln(1+exp(x))
(Softplus has no HW activation table; Exp and Ln share one set).

Outputs per core: weights (per point) and expincl = exp(-interval *
inclusive-scan) per point; the host gathers alphainv_last[r] =
expincl[last point of ray r] (empty rays -> 1.0).
"""

import functools

import numpy as np

P = 128           # SBUF partitions
NTILES = 8        # column tiles per row
N_CORES = 8


@functools.lru_cache(maxsize=4)
def _get_program(ft: int, shift: float, interval: float, rid16: bool):
    import concourse.bacc as bacc
    import concourse.mybir as mybir
    import concourse.tile as tile

    AF = mybir.ActivationFunctionType
    OP = mybir.AluOpType
    f32 = mybir.dt.float32
    ridt = mybir.dt.int16 if rid16 else mybir.dt.int32
    L = ft * NTILES

    nc = bacc.Bacc("TRN2", target_bir_lowering=False, debug=False)
    dens = nc.dram_tensor("density_in", [P, L], f32, kind="ExternalInput")
    rid = nc.dram_tensor("rid_in", [P, L], ridt, kind="ExternalInput")
    wout = nc.dram_tensor("weights_out", [P, L], f32, kind="ExternalOutput")
    eout = nc.dram_tensor("expincl_out", [P, L], f32, kind="ExternalOutput")

    with tile.TileContext(nc) as tc:
        with (
            tc.tile_pool(name="main", bufs=3) as pool,
            tc.tile_pool(name="chain", bufs=2) as chain_pool,
            tc.tile_pool(name="small", bufs=1) as spool,
        ):
            shift_sb = spool.tile([P, 1], f32)
            nc.vector.memset(shift_sb[:], float(shift))

            r_prev = None
            incl_prev = None
            for t in range(NTILES):
                cs = slice(t * ft, (t + 1) * ft)
                d_t = pool.tile([P, ft], f32, tag="d")
                r_t = chain_pool.tile([P, ft], ridt, tag="r")
                nc.sync.dma_start(out=d_t[:], in_=dens[:, cs])
                nc.sync.dma_start(out=r_t[:], in_=rid[:, cs])

                # softplus(x + shift) = ln(1 + exp(x + shift))
                # Exp in place on d (ACT -> ACT, same engine), then Ln to sp
                nc.scalar.activation(out=d_t[:], in_=d_t[:], func=AF.Exp,
                                     bias=shift_sb[:, :1], scale=1.0)
                sp_t = pool.tile([P, ft], f32, tag="sp")
                nc.scalar.activation(out=sp_t[:], in_=d_t[:], func=AF.Ln,
                                     bias=1.0, scale=1.0)

                # continue-mask: 1 where same ray as previous point, 0 at ray starts
                m_t = pool.tile([P, ft], f32, tag="m")
                nc.vector.tensor_tensor(out=m_t[:, 1:], in0=r_t[:, 1:],
                                        in1=r_t[:, : ft - 1], op=OP.is_equal)
                if t == 0:
                    # every row starts at a ray boundary
                    nc.vector.memset(m_t[:, 0:1], 0.0)
                else:
                    nc.vector.tensor_tensor(out=m_t[:, 0:1], in0=r_t[:, 0:1],
                                            in1=r_prev[:, ft - 1:ft],
                                            op=OP.is_equal)

                # segmented inclusive scan: state = m*state + sp
                incl_t = chain_pool.tile([P, ft], f32, tag="incl")
                nc.vector.tensor_tensor_scan(
                    out=incl_t[:], data0=m_t[:], data1=sp_t[:],
                    initial=(0.0 if t == 0 else incl_prev[:, ft - 1:ft]),
                    op0=OP.mult, op1=OP.add,
                )

                # excl = incl - sp  (exclusive scan)
                excl_t = pool.tile([P, ft], f32, tag="excl")
                nc.vector.tensor_sub(out=excl_t[:], in0=incl_t[:], in1=sp_t[:])
                # T = exp(-interval * excl)
                T_t = pool.tile([P, ft], f32, tag="T")
                nc.scalar.activation(out=T_t[:], in_=excl_t[:], func=AF.Exp,
                                     scale=-float(interval))
                # expincl = exp(-interval * incl)
                ei_t = pool.tile([P, ft], f32, tag="ei")
                nc.scalar.activation(out=ei_t[:], in_=incl_t[:], func=AF.Exp,
                                     scale=-float(interval))
                # weights = T - expincl = alpha * T
                w_t = pool.tile([P, ft], f32, tag="w")
                nc.vector.tensor_sub(out=w_t[:], in0=T_t[:], in1=ei_t[:])

                nc.sync.dma_start(out=wout[:, cs], in_=w_t[:])
                nc.sync.dma_start(out=eout[:, cs], in_=ei_t[:])
                r_prev, incl_prev = r_t, incl_t
    nc.compile()
    return nc


def _plan(rid: np.ndarray, n_rays: int, n_pts: int):
    """Host-side sharding: chunk + row boundaries, all snapped to ray starts."""
    rays_per_core = n_rays // N_CORES
    marks = np.arange(1, N_CORES) * rays_per_core
    bounds = np.concatenate(
        [[0], np.searchsorted(rid, marks, side="left"), [n_pts]]
    ).astype(np.int64)

    row_starts = []
    max_len = 0
    for k in range(N_CORES):
        b0, b1 = int(bounds[k]), int(bounds[k + 1])
        n_c = b1 - b0
        targets = b0 + (np.arange(P, dtype=np.int64) * n_c) // P
        ray_of_target = rid[np.minimum(targets, n_pts - 1)]
        s = np.searchsorted(rid, ray_of_target, side="left").astype(np.int64)
        s[0] = b0
        lens = np.diff(np.append(s, b1))
        max_len = max(max_len, int(lens.max()))
        row_starts.append(s)
    return bounds, row_starts, max_len


def _grid_shapes(rid, n_rays, n_pts):
    bounds, row_starts, max_len = _plan(rid, n_rays, n_pts)
    ft = -(-max_len // NTILES)
    ft = -(-ft // 32) * 32
    return bounds, row_starts, ft


def _make_in_maps(density, rid, bounds, row_starts, ft, rid16):
    L = ft * NTILES
    in_maps = []
    for k in range(N_CORES):
        b0, b1 = int(bounds[k]), int(bounds[k + 1])
        s = row_starts[k]
        lens = np.diff(np.append(s, b1))
        d_grid = np.zeros((P, L), np.float32)
        r_grid = np.zeros((P, L), np.int16 if rid16 else np.int32)
        for p in range(P):
            n = int(lens[p])
            if n:
                d_grid[p, :n] = density[s[p]:s[p] + n]
                seg = rid[s[p]:s[p] + n]
                if rid16:
                    seg = seg.astype(np.uint16).view(np.int16)
                r_grid[p, :n] = seg
        in_maps.append({"density_in": d_grid, "rid_in": r_grid})
    return in_maps


def kernel(density, shift, interval, ray_id, n_rays):
    from concourse.bass_utils import run_bass_kernel_spmd

    density = np.ascontiguousarray(np.asarray(density), dtype=np.float32).ravel()
    rid = np.ascontiguousarray(np.asarray(ray_id), dtype=np.int32).ravel()
    shift_f = float(np.asarray(shift))
    interval_f = float(np.asarray(interval))
    n_rays_i = int(np.asarray(n_rays))
    n_pts = density.shape[0]
    assert rid.shape[0] == n_pts and n_rays_i % N_CORES == 0

    bounds, row_starts, ft = _grid_shapes(rid, n_rays_i, n_pts)
    rid16 = n_rays_i <= 65536
    nc = _get_program(ft, shift_f, interval_f, rid16)
    in_maps = _make_in_maps(density, rid, bounds, row_starts, ft, rid16)

    res = run_bass_kernel_spmd(nc, in_maps, list(range(N_CORES))).results

    weights = np.empty(n_pts, np.float32)
    expincl = np.empty(n_pts, np.float32)
    for k in range(N_CORES):
        b0, b1 = int(bounds[k]), int(bounds[k + 1])
        s = row_starts[k]
        lens = np.diff(np.append(s, b1))
        w_grid = res[k]["weights_out"]
        e_grid = res[k]["expincl_out"]
        for p in range(P):
            n = int(lens[p])
            if n:
                weights[s[p]:s[p] + n] = w_grid[p, :n]
                expincl[s[p]:s[p] + n] = e_grid[p, :n]

    ray_ids = np.arange(n_rays_i)
    ends = np.searchsorted(rid, ray_ids, side="right") - 1
    starts = np.searchsorted(rid, ray_ids, side="left")
    alphainv_last = np.ones(n_rays_i, np.float32)
    nonempty = ends >= starts
    alphainv_last[nonempty] = expincl[ends[nonempty]]
    return weights, alphainv_last


# revision 10
# speedup vs baseline: 1770.2514x; 1.4035x over previous
"""DirectVoxGO Raw2Alpha + Alphas2Weights (segmented scan) on 8 Trainium2 cores.

Computes, for n_pts sample points sorted by ray_id:
    sp      = softplus(density + shift)
    log(1-alpha) = -interval * sp
    weights = alpha * T   (T = exclusive per-ray cumprod of (1-alpha))
    alphainv_last[r] = prod over ray r of (1-alpha)

Strategy
--------
Shard by ray: each of the 8 cores gets a contiguous chunk of points that
covers a contiguous range of rays (chunk boundaries snapped to ray starts,
host-side searchsorted).  Inside a core the chunk is laid out as a
[128, L] grid; each partition row also starts at a ray boundary (host
marshals rows into the grid, padding short rows).  Because every row
starts at a ray start, the per-ray segmented scan never crosses a
partition boundary and a single tensor_tensor_scan pass per column tile
computes it:   state = m * state + sp    (m = 0 at ray starts).

The scan is linear in its data input, so the -interval factor is folded
into the Exp activations' scale.  softplus is computed as ln(1+exp(x))
(Softplus has no HW activation table; Exp and Ln share one set).

Outputs per core: weights (per point) and expincl = exp(-interval *
inclusive-scan) per point; the host gathers alphainv_last[r] =
expincl[last point of ray r] (empty rays -> 1.0).
"""

import functools

import numpy as np

P = 128           # SBUF partitions
NTILES = 8        # column tiles per row
N_CORES = 8


@functools.lru_cache(maxsize=4)
def _get_program(ft: int, shift: float, interval: float, rid16: bool):
    import concourse.bacc as bacc
    import concourse.mybir as mybir
    import concourse.tile as tile

    AF = mybir.ActivationFunctionType
    OP = mybir.AluOpType
    f32 = mybir.dt.float32
    ridt = mybir.dt.int16 if rid16 else mybir.dt.int32
    L = ft * NTILES

    nc = bacc.Bacc("TRN2", target_bir_lowering=False, debug=False)
    dens = nc.dram_tensor("density_in", [P, L], f32, kind="ExternalInput")
    rid = nc.dram_tensor("rid_in", [P, L], ridt, kind="ExternalInput")
    wout = nc.dram_tensor("weights_out", [P, L], f32, kind="ExternalOutput")
    eout = nc.dram_tensor("expincl_out", [P, L], f32, kind="ExternalOutput")

    with tile.TileContext(nc) as tc:
        with (
            tc.tile_pool(name="main", bufs=3) as pool,
            tc.tile_pool(name="load", bufs=3) as lpool,
            tc.tile_pool(name="chain", bufs=2) as chain_pool,
            tc.tile_pool(name="small", bufs=1) as spool,
        ):
            shift_sb = spool.tile([P, 1], f32)
            nc.vector.memset(shift_sb[:], float(shift))

            incl_prev = None
            for t in range(NTILES):
                cs = slice(t * ft, (t + 1) * ft)
                # loads on SP (HWDGE): no data deps, never block the queue
                d_t = lpool.tile([P, ft], f32, tag="d")
                nc.sync.dma_start(out=d_t[:], in_=dens[:, cs])
                # rid tile loaded with one leading overlap column so the
                # whole continue-mask is one shifted compare (no cross-tile
                # chaining).  Tile 0's column -1 is a -1 sentinel; its mask
                # value is irrelevant because the scan's initial state is 0.
                r_t = lpool.tile([P, ft + 1], ridt, tag="r")
                if t == 0:
                    nc.vector.memset(r_t[:, 0:1], -1)
                    nc.sync.dma_start(out=r_t[:, 1:], in_=rid[:, 0:ft])
                else:
                    nc.sync.dma_start(out=r_t[:],
                                      in_=rid[:, t * ft - 1:(t + 1) * ft])

                # softplus(x + shift) = ln(1 + exp(x + shift))
                # Exp in place on d (ACT -> ACT, same engine), then Ln to sp
                nc.scalar.activation(out=d_t[:], in_=d_t[:], func=AF.Exp,
                                     bias=shift_sb[:, :1], scale=1.0)
                sp_t = pool.tile([P, ft], f32, tag="sp")
                nc.scalar.activation(out=sp_t[:], in_=d_t[:], func=AF.Ln,
                                     bias=1.0, scale=1.0)

                # continue-mask: 1 where same ray as previous point, 0 at ray
                # starts (row starts are ray starts by construction)
                m_t = pool.tile([P, ft], f32, tag="m")
                nc.vector.tensor_tensor(out=m_t[:], in0=r_t[:, 1:],
                                        in1=r_t[:, :ft], op=OP.is_equal)

                # segmented inclusive scan: state = m*state + sp
                incl_t = chain_pool.tile([P, ft], f32, tag="incl")
                nc.vector.tensor_tensor_scan(
                    out=incl_t[:], data0=m_t[:], data1=sp_t[:],
                    initial=(0.0 if t == 0 else incl_prev[:, ft - 1:ft]),
                    op0=OP.mult, op1=OP.add,
                )

                # excl = incl - sp  (exclusive scan)
                excl_t = pool.tile([P, ft], f32, tag="excl")
                nc.vector.tensor_sub(out=excl_t[:], in0=incl_t[:], in1=sp_t[:])
                # T = exp(-interval * excl)
                T_t = pool.tile([P, ft], f32, tag="T")
                nc.scalar.activation(out=T_t[:], in_=excl_t[:], func=AF.Exp,
                                     scale=-float(interval))
                # expincl = exp(-interval * incl)
                ei_t = pool.tile([P, ft], f32, tag="ei")
                nc.scalar.activation(out=ei_t[:], in_=incl_t[:], func=AF.Exp,
                                     scale=-float(interval))
                # weights = T - expincl = alpha * T
                w_t = pool.tile([P, ft], f32, tag="w")
                nc.vector.tensor_sub(out=w_t[:], in0=T_t[:], in1=ei_t[:])

                # stores on Pool (SWDGE): their data-waits don't block loads
                nc.gpsimd.dma_start(out=wout[:, cs], in_=w_t[:])
                nc.gpsimd.dma_start(out=eout[:, cs], in_=ei_t[:])
                incl_prev = incl_t

    # Compile with only activation tables containing BOTH Exp and Ln
    # offered, so every activation resolves to one table set (otherwise the
    # chooser alternates sets per tile, costing an ~2.7us table DMA each
    # time).  Scoped + restored: only affects this compile.
    AFt = mybir.ActivationFunctionType
    orig_tables = bacc.get_activation_tables

    def one_table(arch, _orig=orig_tables, _AF=AFt):
        tabs = _orig(arch)
        if not any(_AF.Exp in s and _AF.Ln in s for s in tabs.values()):
            return tabs
        # Keep every entry at its original position (the pass encodes the
        # table id positionally); empty the sets we don't want chosen.
        return {n: (s if (_AF.Exp in s and _AF.Ln in s) else set())
                for n, s in tabs.items()}

    bacc.get_activation_tables = one_table
    try:
        nc.compile()
    finally:
        bacc.get_activation_tables = orig_tables
    return nc


def _plan(rid: np.ndarray, n_rays: int, n_pts: int):
    """Host-side sharding: chunk + row boundaries, all snapped to ray starts."""
    rays_per_core = n_rays // N_CORES
    marks = np.arange(1, N_CORES) * rays_per_core
    bounds = np.concatenate(
        [[0], np.searchsorted(rid, marks, side="left"), [n_pts]]
    ).astype(np.int64)

    row_starts = []
    max_len = 0
    for k in range(N_CORES):
        b0, b1 = int(bounds[k]), int(bounds[k + 1])
        n_c = b1 - b0
        targets = b0 + (np.arange(P, dtype=np.int64) * n_c) // P
        ray_of_target = rid[np.minimum(targets, n_pts - 1)]
        s = np.searchsorted(rid, ray_of_target, side="left").astype(np.int64)
        s[0] = b0
        lens = np.diff(np.append(s, b1))
        max_len = max(max_len, int(lens.max()))
        row_starts.append(s)
    return bounds, row_starts, max_len


def _grid_shapes(rid, n_rays, n_pts):
    bounds, row_starts, max_len = _plan(rid, n_rays, n_pts)
    ft = -(-max_len // NTILES)
    ft = -(-ft // 32) * 32
    return bounds, row_starts, ft


def _make_in_maps(density, rid, bounds, row_starts, ft, rid16):
    L = ft * NTILES
    in_maps = []
    for k in range(N_CORES):
        b0, b1 = int(bounds[k]), int(bounds[k + 1])
        s = row_starts[k]
        lens = np.diff(np.append(s, b1))
        d_grid = np.zeros((P, L), np.float32)
        r_grid = np.zeros((P, L), np.int16 if rid16 else np.int32)
        for p in range(P):
            n = int(lens[p])
            if n:
                d_grid[p, :n] = density[s[p]:s[p] + n]
                seg = rid[s[p]:s[p] + n]
                if rid16:
                    seg = seg.astype(np.uint16).view(np.int16)
                r_grid[p, :n] = seg
        in_maps.append({"density_in": d_grid, "rid_in": r_grid})
    return in_maps


def kernel(density, shift, interval, ray_id, n_rays):
    from concourse.bass_utils import run_bass_kernel_spmd

    density = np.ascontiguousarray(np.asarray(density), dtype=np.float32).ravel()
    rid = np.ascontiguousarray(np.asarray(ray_id), dtype=np.int32).ravel()
    shift_f = float(np.asarray(shift))
    interval_f = float(np.asarray(interval))
    n_rays_i = int(np.asarray(n_rays))
    n_pts = density.shape[0]
    assert rid.shape[0] == n_pts and n_rays_i % N_CORES == 0

    bounds, row_starts, ft = _grid_shapes(rid, n_rays_i, n_pts)
    rid16 = n_rays_i <= 65536
    nc = _get_program(ft, shift_f, interval_f, rid16)
    in_maps = _make_in_maps(density, rid, bounds, row_starts, ft, rid16)

    res = run_bass_kernel_spmd(nc, in_maps, list(range(N_CORES))).results

    weights = np.empty(n_pts, np.float32)
    expincl = np.empty(n_pts, np.float32)
    for k in range(N_CORES):
        b0, b1 = int(bounds[k]), int(bounds[k + 1])
        s = row_starts[k]
        lens = np.diff(np.append(s, b1))
        w_grid = res[k]["weights_out"]
        e_grid = res[k]["expincl_out"]
        for p in range(P):
            n = int(lens[p])
            if n:
                weights[s[p]:s[p] + n] = w_grid[p, :n]
                expincl[s[p]:s[p] + n] = e_grid[p, :n]

    ray_ids = np.arange(n_rays_i)
    ends = np.searchsorted(rid, ray_ids, side="right") - 1
    starts = np.searchsorted(rid, ray_ids, side="left")
    alphainv_last = np.ones(n_rays_i, np.float32)
    nonempty = ends >= starts
    alphainv_last[nonempty] = expincl[ends[nonempty]]
    return weights, alphainv_last
